# revision 7
# baseline (speedup 1.0000x reference)
"""Multi-head attention with bias, distributed over 8 trn2 NeuronCores.

Reference computation (per batch b):
    q = (x @ Wq.T) * depth**-0.5 ; k = y @ Wk.T ; v = y @ Wv.T     (per-head split)
    out = softmax(q @ k.T + bias) @ v @ Wo.T

Sharding: 8 cores = 4 batches x 2 query-row halves.  Core c handles batch
b = c//2 and query rows (c%2)*1024 .. +1024.  k/v projections are computed
redundantly inside each pair (25% extra flops) so there are NO collectives.

Device-side layout (everything "transposed", feature dim on partitions):
    qT/kT = W.T-projected activations [d_out, seq]; v natural [seq, d_out].
    logitsT[kk, i] = kT_h-slice.T @ qT_h-slice  (K=64 contraction)
    expw = exp(logitsT) * exp(bias).T           (exp(bias) precomputed on host)
    attnT_h(+denom row) = [v_h | ones].T @ expw (K=128, denom rides as row 64)
    normalize via batched DVE reciprocal + DMA partition-broadcast from DRAM
    outT = Wo.T-proj of normalized attnT.
Host does: transposes, bf16 casts, exp(bias), scale fold into Wq.
"""

import numpy as np
import ml_dtypes
from contextlib import ExitStack

import concourse.bass as bass
import concourse.mybir as mybir
import concourse.tile as tile
from concourse import bacc
from concourse.bass_utils import run_bass_kernel_spmd

# full-problem dims (hardcoded per spec)
B, S, D, H = 4, 2048, 1024, 16
DEPTH = D // H            # 64
P = 128
NCORES = 8

BF = mybir.dt.bfloat16
F32 = mybir.dt.float32
EXP = mybir.ActivationFunctionType.Exp

TRACE = False
last_exec_time_ns = None
last_results = None


def _chunks(total, step):
    return [(n0, min(n0 + step, total)) for n0 in range(0, total, step)]


def _attn_body(ctx, tc, io, S_, D_, H_, SL_):
    """Emit the per-core kernel.  S_: kv seq len, SL_: q rows on this core."""
    nc = tc.nc
    NT = D_ // P              # d tiles
    KT = S_ // P              # kk tiles
    HPT = P // DEPTH          # heads per d-tile = 2
    xT, yT, ebT, wqT, wkT, wvT, woT, outT = (
        io[k] for k in ("xT", "yT", "ebT", "wqT", "wkT", "wvT", "woT", "outT"))

    # pools that live the whole kernel
    qpool = ctx.enter_context(tc.tile_pool(name="qpool", bufs=NT))
    kpool = ctx.enter_context(tc.tile_pool(name="kpool", bufs=NT))
    vpool = ctx.enter_context(tc.tile_pool(name="vpool", bufs=KT))
    plp = ctx.enter_context(tc.tile_pool(name="plp", bufs=2, space="PSUM"))
    pap = ctx.enter_context(tc.tile_pool(name="pap", bufs=2, space="PSUM"))
    dpool = ctx.enter_context(tc.tile_pool(name="dpool", bufs=1, space="DRAM"))

    q_sb = [qpool.tile([P, SL_], BF, tag="qT", name=f"q{m}", bufs=NT)
            for m in range(NT)]
    k_sb = [kpool.tile([P, S_], BF, tag="kT", name=f"k{m}", bufs=NT)
            for m in range(NT)]
    v_sb = [vpool.tile([P, H_, 66], BF, tag="v66", name=f"v{c}", bufs=KT)
            for c in range(KT)]

    # ---------------- projections (phase-scoped pools) ----------------
    with tc.tile_pool(name="ypool", bufs=NT) as ypool, \
         tc.tile_pool(name="xpool", bufs=NT) as xpool, \
         tc.tile_pool(name="wpool", bufs=2 * NT) as wpool:
        y_sb = [ypool.tile([P, S_], BF, tag="yT", name=f"y{t}", bufs=NT)
                for t in range(NT)]
        for t in range(NT):
            nc.sync.dma_start(out=y_sb[t], in_=yT[t * P:(t + 1) * P, :])
        wv_sb = [wpool.tile([P, D_], BF, tag="w", name=f"wv{t}", bufs=2 * NT)
                 for t in range(NT)]
        for t in range(NT):
            nc.sync.dma_start(out=wv_sb[t], in_=wvT[t * P:(t + 1) * P, :])

        # v in natural layout [kk, head, 66]: cols 0-63 data, 64 ones, 65 pad
        for c in range(KT):
            vt = v_sb[c]
            nc.vector.memset(vt[:, :, 64:65], 1.0)
            nc.vector.memset(vt[:, :, 65:66], 0.0)
            for gi, (n0, n1) in enumerate(_chunks(D_, 512)):
                ps = plp.tile([P, 1024], F32, tag="pl", name=f"psv{c}_{gi}",
                              bufs=2)
                for t in range(NT):
                    nc.tensor.matmul(ps[:, 0:n1 - n0],
                                     lhsT=y_sb[t][:, c * P:(c + 1) * P],
                                     rhs=wv_sb[t][:, n0:n1],
                                     start=(t == 0), stop=(t == NT - 1))
                ng = (n1 - n0) // DEPTH
                src = ps[:, 0:n1 - n0].rearrange("p (g d) -> p g d", d=DEPTH)
                dst = vt[:, gi * ng:(gi + 1) * ng, 0:DEPTH]
                nc.vector.tensor_copy(dst, src)

        x_sb = [xpool.tile([P, SL_], BF, tag="xT", name=f"x{t}", bufs=NT)
                for t in range(NT)]
        for t in range(NT):
            nc.sync.dma_start(out=x_sb[t], in_=xT[t * P:(t + 1) * P, :])
        wq_sb = [wpool.tile([P, D_], BF, tag="w", name=f"wq{t}", bufs=2 * NT)
                 for t in range(NT)]
        for t in range(NT):
            nc.sync.dma_start(out=wq_sb[t], in_=wqT[t * P:(t + 1) * P, :])

        for m in range(NT):
            for n0, n1 in _chunks(SL_, 512):
                ps = plp.tile([P, 1024], F32, tag="pl", name=f"psq{m}_{n0}",
                              bufs=2)
                for t in range(NT):
                    nc.tensor.matmul(ps[:, 0:n1 - n0],
                                     lhsT=wq_sb[t][:, m * P:(m + 1) * P],
                                     rhs=x_sb[t][:, n0:n1],
                                     start=(t == 0), stop=(t == NT - 1))
                nc.vector.tensor_copy(q_sb[m][:, n0:n1], ps[:, 0:n1 - n0])

        wk_sb = [wpool.tile([P, D_], BF, tag="w", name=f"wk{t}", bufs=2 * NT)
                 for t in range(NT)]
        for t in range(NT):
            nc.sync.dma_start(out=wk_sb[t], in_=wkT[t * P:(t + 1) * P, :])
        for m in range(NT):
            for n0, n1 in _chunks(S_, 512):
                ps = plp.tile([P, 1024], F32, tag="pl", name=f"psk{m}_{n0}",
                              bufs=2)
                for t in range(NT):
                    nc.tensor.matmul(ps[:, 0:n1 - n0],
                                     lhsT=wk_sb[t][:, m * P:(m + 1) * P],
                                     rhs=y_sb[t][:, n0:n1],
                                     start=(t == 0), stop=(t == NT - 1))
                nc.vector.tensor_copy(k_sb[m][:, n0:n1], ps[:, 0:n1 - n0])

    # ---------------- attention (own pools) ----------------
    ebpool = ctx.enter_context(tc.tile_pool(name="ebpool", bufs=KT))
    epool = ctx.enter_context(tc.tile_pool(name="epool", bufs=6))
    aupool = ctx.enter_context(tc.tile_pool(name="aupool", bufs=NT))
    stpool = ctx.enter_context(tc.tile_pool(name="stpool", bufs=5))
    smpool = ctx.enter_context(tc.tile_pool(name="smpool", bufs=4))
    opool = ctx.enter_context(tc.tile_pool(name="opool", bufs=2))
    wopool = ctx.enter_context(tc.tile_pool(name="wopool", bufs=NT))

    eb_sb = [ebpool.tile([P, SL_], BF, tag="eb", name=f"eb{c}", bufs=KT)
             for c in range(KT)]
    for c in range(KT):
        nc.sync.dma_start(out=eb_sb[c], in_=ebT[c * P:(c + 1) * P, :])

    # unnormalized attnT, assembled to full 128-partition tiles via DMA
    au_sb = [aupool.tile([P, SL_], BF, tag="au", name=f"au{t}", bufs=NT)
             for t in range(NT)]
    den_sb = smpool.tile([H_, SL_], BF, tag="den", name="den", bufs=1)

    for t in range(NT):
        for half in range(HPT):
            h = HPT * t + half
            r0 = half * DEPTH
            pattn = pap.tile([65, SL_], F32, tag="pattn", name=f"pa{h}", bufs=2)
            for c in range(KT):
                plt = plp.tile([P, 1024], F32, tag="pl", name=f"pl{h}_{c}",
                               bufs=2)
                for n0, n1 in _chunks(SL_, 512):
                    nc.tensor.matmul(plt[:, n0:n1],
                                     lhsT=k_sb[t][r0:r0 + DEPTH,
                                                  c * P:(c + 1) * P],
                                     rhs=q_sb[t][r0:r0 + DEPTH, n0:n1],
                                     start=True, stop=True)
                ew = epool.tile([P, SL_], BF, tag="ew", name=f"ew{h}_{c}",
                                bufs=3)
                nc.scalar.activation(ew, plt[:, 0:SL_], EXP)
                ew2 = epool.tile([P, SL_], BF, tag="ew2", name=f"ew2{h}_{c}",
                                 bufs=3)
                nc.vector.tensor_mul(ew2, ew, eb_sb[c])
                for n0, n1 in _chunks(SL_, 512):
                    nc.tensor.matmul(pattn[:, n0:n1],
                                     lhsT=v_sb[c][:, h, 0:65],
                                     rhs=ew2[:, n0:n1],
                                     start=(c == 0), stop=(c == KT - 1))
            # denominator row lives at psum partition 64: stage (partition-
            # preserving DVE copy) then SBUF->SBUF DMA into den_sb row h
            stg = stpool.tile([P, SL_], BF, tag="stg", name=f"st{h}", bufs=2)
            nc.vector.tensor_copy(stg[64:65, :], pattn[64:65, :])
            nc.sync.dma_start(out=den_sb[h:h + 1, :], in_=stg[64:65, :])
            # unnormalized attn: DVE to sbuf (base 0) then DMA to row half
            sau = stpool.tile([DEPTH, SL_], BF, tag="sau", name=f"sa{h}",
                              bufs=3)
            nc.vector.tensor_copy(sau, pattn[0:64, :])
            nc.sync.dma_start(out=au_sb[t][r0:r0 + DEPTH, :], in_=sau)

    # ---------------- normalize (in place on au tiles) ----------------
    recipb = smpool.tile([H_, SL_], BF, tag="recip", name="recipb", bufs=1)
    with nc.allow_low_precision(reason="softmax denom reciprocal in bf16"):
        nc.vector.reciprocal(recipb, den_sb)
    # bounce to DRAM: SBUF sources cannot be partition-broadcast, DRAM can
    rscr = dpool.tile([H_, SL_], BF, tag="rscr", name="rscr", bufs=1)
    nc.sync.dma_start(out=rscr, in_=recipb)
    for t in range(NT):
        bc = smpool.tile([P, SL_], BF, tag="bc", name=f"bc{t}", bufs=2)
        for half in range(HPT):
            h = HPT * t + half
            r0 = half * DEPTH
            nc.sync.dma_start(
                out=bc[r0:r0 + DEPTH, :],
                in_=rscr[h:h + 1, :].partition_broadcast(DEPTH))
        nc.vector.tensor_mul(au_sb[t], au_sb[t], bc)

    # ---------------- output projection ----------------
    wo_sb = [wopool.tile([P, D_], BF, tag="wo", name=f"wo{t}", bufs=NT)
             for t in range(NT)]
    for t in range(NT):
        nc.sync.dma_start(out=wo_sb[t], in_=woT[t * P:(t + 1) * P, :])
    for m in range(NT):
        osb = opool.tile([P, SL_], F32, tag="osb", name=f"o{m}", bufs=2)
        for n0, n1 in _chunks(SL_, 512):
            ps = plp.tile([P, 1024], F32, tag="pl", name=f"pso{m}_{n0}", bufs=2)
            for t in range(NT):
                nc.tensor.matmul(ps[:, 0:n1 - n0],
                                 lhsT=wo_sb[t][:, m * P:(m + 1) * P],
                                 rhs=au_sb[t][:, n0:n1],
                                 start=(t == 0), stop=(t == NT - 1))
            nc.vector.tensor_copy(osb[:, n0:n1], ps[:, 0:n1 - n0])
        nc.sync.dma_start(out=outT[m * P:(m + 1) * P, :], in_=osb)


def build_nc(S_=S, D_=D, H_=H, SL_=None):
    if SL_ is None:
        SL_ = S_ // 2
    nc = bacc.Bacc("TRN2", target_bir_lowering=False, debug=False)
    io = {
        "xT": nc.dram_tensor("xT", [D_, SL_], BF, kind="ExternalInput").ap(),
        "yT": nc.dram_tensor("yT", [D_, S_], BF, kind="ExternalInput").ap(),
        "ebT": nc.dram_tensor("ebT", [S_, SL_], BF, kind="ExternalInput").ap(),
        "wqT": nc.dram_tensor("wqT", [D_, D_], BF, kind="ExternalInput").ap(),
        "wkT": nc.dram_tensor("wkT", [D_, D_], BF, kind="ExternalInput").ap(),
        "wvT": nc.dram_tensor("wvT", [D_, D_], BF, kind="ExternalInput").ap(),
        "woT": nc.dram_tensor("woT", [D_, D_], BF, kind="ExternalInput").ap(),
        "outT": nc.dram_tensor("outT", [D_, SL_], F32,
                               kind="ExternalOutput").ap(),
    }
    with tile.TileContext(nc) as tc:
        with ExitStack() as ctx:
            _attn_body(ctx, tc, io, S_, D_, H_, SL_)
    nc.compile()
    return nc


_NC_CACHE = None


def kernel(x, y, bias, Wq, Wk, Wv, Wo):
    global _NC_CACHE, last_exec_time_ns, last_results
    x = np.asarray(x, np.float32)
    y = np.asarray(y, np.float32)
    bias = np.asarray(bias, np.float32)
    Wq, Wk, Wv, Wo = (np.asarray(w, np.float32) for w in (Wq, Wk, Wv, Wo))
    SL_ = S // 2
    if _NC_CACHE is None:
        _NC_CACHE = build_nc()
    nc = _NC_CACHE

    bf = ml_dtypes.bfloat16
    scale = DEPTH ** -0.5
    wqT = np.ascontiguousarray(Wq.T * scale).astype(bf)
    wkT = np.ascontiguousarray(Wk.T).astype(bf)
    wvT = np.ascontiguousarray(Wv.T).astype(bf)
    woT = np.ascontiguousarray(Wo.T).astype(bf)
    eb = np.exp(bias[0, 0].astype(np.float32))
    ebT_half = [np.ascontiguousarray(eb[q0:q0 + SL_, :].T).astype(bf)
                for q0 in (0, SL_)]
    yT_all = [np.ascontiguousarray(y[b].T).astype(bf) for b in range(B)]

    in_maps = []
    for core in range(NCORES):
        b, half = divmod(core, 2)
        qs = half * SL_
        in_maps.append({
            "xT": np.ascontiguousarray(x[b, qs:qs + SL_, :].T).astype(bf),
            "yT": yT_all[b],
            "ebT": ebT_half[half],
            "wqT": wqT, "wkT": wkT, "wvT": wvT, "woT": woT,
        })

    res = run_bass_kernel_spmd(nc, in_maps, core_ids=list(range(NCORES)),
                               trace=TRACE)
    last_exec_time_ns = res.exec_time_ns
    last_results = res
    out = np.empty((B, S, D), np.float32)
    for core in range(NCORES):
        b, half = divmod(core, 2)
        qs = half * SL_
        out[b, qs:qs + SL_, :] = res.results[core]["outT"].T
    return out


# revision 10
# speedup vs baseline: 1.3946x; 1.3946x over previous
"""Multi-head attention with bias, distributed over 8 trn2 NeuronCores.

Reference computation (per batch b):
    q = (x @ Wq.T) * depth**-0.5 ; k = y @ Wk.T ; v = y @ Wv.T     (per-head split)
    out = softmax(q @ k.T + bias) @ v @ Wo.T

Sharding: 8 cores = 4 batches x 2 query-row halves.  Core c handles batch
b = c//2 and query rows (c%2)*1024 .. +1024.  k/v projections are computed
redundantly inside each pair (25% extra flops) so there are NO collectives.

Device-side layout (everything "transposed", feature dim on partitions):
    qT/kT = W.T-projected activations [d_out, seq]; v natural [seq, d_out].
    logitsT[kk, i] = kT_h-slice.T @ qT_h-slice  (K=64 contraction)
    expw = exp(logitsT) * exp(bias).T           (exp(bias) precomputed on host)
    attnT_h(+denom row) = [v_h | ones].T @ expw (K=128, denom rides as row 64)
    normalize via batched DVE reciprocal + DMA partition-broadcast from DRAM
    outT = Wo.T-proj of normalized attnT.
Host does: transposes, bf16 casts, exp(bias), scale fold into Wq.
"""

import numpy as np
import ml_dtypes
from contextlib import ExitStack

import concourse.bass as bass
import concourse.mybir as mybir
import concourse.tile as tile
from concourse import bacc
from concourse.bass_utils import run_bass_kernel_spmd

# full-problem dims (hardcoded per spec)
B, S, D, H = 4, 2048, 1024, 16
DEPTH = D // H            # 64
P = 128
NCORES = 8

BF = mybir.dt.bfloat16
F32 = mybir.dt.float32
EXP = mybir.ActivationFunctionType.Exp

TRACE = False
last_exec_time_ns = None
last_results = None


def _chunks(total, step):
    return [(n0, min(n0 + step, total)) for n0 in range(0, total, step)]


def _attn_body(ctx, tc, io, S_, D_, H_, SL_):
    """Emit the per-core kernel.  S_: kv seq len, SL_: q rows on this core."""
    nc = tc.nc
    NT = D_ // P              # d tiles
    KT = S_ // P              # kk tiles
    HPT = P // DEPTH          # heads per d-tile = 2
    xT, yT, ebT, wqT, wkT, wvT, woT, outT = (
        io[k] for k in ("xT", "yT", "ebT", "wqT", "wkT", "wvT", "woT", "outT"))

    # pools that live the whole kernel
    ebpool = ctx.enter_context(tc.tile_pool(name="ebpool", bufs=KT))
    qpool = ctx.enter_context(tc.tile_pool(name="qpool", bufs=NT))
    kpool = ctx.enter_context(tc.tile_pool(name="kpool", bufs=NT))
    vpool = ctx.enter_context(tc.tile_pool(name="vpool", bufs=KT))
    plp = ctx.enter_context(tc.tile_pool(name="plp", bufs=2, space="PSUM"))
    pap = ctx.enter_context(tc.tile_pool(name="pap", bufs=2, space="PSUM"))
    dpool = ctx.enter_context(tc.tile_pool(name="dpool", bufs=1, space="DRAM"))

    q_sb = [qpool.tile([P, SL_], BF, tag="qT", name=f"q{m}", bufs=NT)
            for m in range(NT)]
    k_sb = [kpool.tile([P, S_], BF, tag="kT", name=f"k{m}", bufs=NT)
            for m in range(NT)]
    v_sb = [vpool.tile([P, H_, 66], BF, tag="v66", name=f"v{c}", bufs=KT)
            for c in range(KT)]

    # ---------------- projections (phase-scoped pools) ----------------
    with tc.tile_pool(name="ypool", bufs=NT) as ypool, \
         tc.tile_pool(name="xpool", bufs=NT) as xpool, \
         tc.tile_pool(name="wpool", bufs=2 * NT) as wpool:
        y_sb = [ypool.tile([P, S_], BF, tag="yT", name=f"y{t}", bufs=NT)
                for t in range(NT)]
        for t in range(NT):
            nc.sync.dma_start(out=y_sb[t], in_=yT[t * P:(t + 1) * P, :])
        wv_sb = [wpool.tile([P, D_], BF, tag="w", name=f"wv{t}", bufs=2 * NT)
                 for t in range(NT)]
        for t in range(NT):
            nc.sync.dma_start(out=wv_sb[t], in_=wvT[t * P:(t + 1) * P, :])

        # v in natural layout [kk, head, 66]: cols 0-63 data, 64 ones, 65 pad
        for c in range(KT):
            vt = v_sb[c]
            nc.vector.memset(vt[:, :, 64:65], 1.0)
            nc.vector.memset(vt[:, :, 65:66], 0.0)
            for gi, (n0, n1) in enumerate(_chunks(D_, 512)):
                ps = plp.tile([P, 1024], F32, tag="pl", name=f"psv{c}_{gi}",
                              bufs=2)
                for t in range(NT):
                    nc.tensor.matmul(ps[:, 0:n1 - n0],
                                     lhsT=y_sb[t][:, c * P:(c + 1) * P],
                                     rhs=wv_sb[t][:, n0:n1],
                                     start=(t == 0), stop=(t == NT - 1))
                ng = (n1 - n0) // DEPTH
                src = ps[:, 0:n1 - n0].rearrange("p (g d) -> p g d", d=DEPTH)
                dst = vt[:, gi * ng:(gi + 1) * ng, 0:DEPTH]
                nc.vector.tensor_copy(dst, src)

        x_sb = [xpool.tile([P, SL_], BF, tag="xT", name=f"x{t}", bufs=NT)
                for t in range(NT)]
        for t in range(NT):
            nc.sync.dma_start(out=x_sb[t], in_=xT[t * P:(t + 1) * P, :])
        wq_sb = [wpool.tile([P, D_], BF, tag="w", name=f"wq{t}", bufs=2 * NT)
                 for t in range(NT)]
        for t in range(NT):
            nc.sync.dma_start(out=wq_sb[t], in_=wqT[t * P:(t + 1) * P, :])

        for m in range(NT):
            for n0, n1 in _chunks(SL_, 512):
                ps = plp.tile([P, 1024], F32, tag="pl", name=f"psq{m}_{n0}",
                              bufs=2)
                for t in range(NT):
                    nc.tensor.matmul(ps[:, 0:n1 - n0],
                                     lhsT=wq_sb[t][:, m * P:(m + 1) * P],
                                     rhs=x_sb[t][:, n0:n1],
                                     start=(t == 0), stop=(t == NT - 1))
                nc.vector.tensor_copy(q_sb[m][:, n0:n1], ps[:, 0:n1 - n0])

        # prefetch exp(bias) tiles during the k projection so the PE has no
        # idle window at the projection->attention boundary (HAM stays warm)
        eb_sb = [ebpool.tile([P, SL_], BF, tag="eb", name=f"eb{c}", bufs=KT)
                 for c in range(KT)]
        for c in range(KT):
            nc.sync.dma_start(out=eb_sb[c], in_=ebT[c * P:(c + 1) * P, :])

        wk_sb = [wpool.tile([P, D_], BF, tag="w", name=f"wk{t}", bufs=2 * NT)
                 for t in range(NT)]
        for t in range(NT):
            nc.sync.dma_start(out=wk_sb[t], in_=wkT[t * P:(t + 1) * P, :])
        for m in range(NT):
            for n0, n1 in _chunks(S_, 512):
                ps = plp.tile([P, 1024], F32, tag="pl", name=f"psk{m}_{n0}",
                              bufs=2)
                for t in range(NT):
                    nc.tensor.matmul(ps[:, 0:n1 - n0],
                                     lhsT=wk_sb[t][:, m * P:(m + 1) * P],
                                     rhs=y_sb[t][:, n0:n1],
                                     start=(t == 0), stop=(t == NT - 1))
                nc.vector.tensor_copy(k_sb[m][:, n0:n1], ps[:, 0:n1 - n0])

    # ---------------- attention (own pools) ----------------
    epool = ctx.enter_context(tc.tile_pool(name="epool", bufs=6))
    aupool = ctx.enter_context(tc.tile_pool(name="aupool", bufs=NT))
    stpool = ctx.enter_context(tc.tile_pool(name="stpool", bufs=5))
    smpool = ctx.enter_context(tc.tile_pool(name="smpool", bufs=4))
    opool = ctx.enter_context(tc.tile_pool(name="opool", bufs=2))
    wopool = ctx.enter_context(tc.tile_pool(name="wopool", bufs=NT))

    # prefetch Wo now; it is consumed only in the final phase
    wo_sb = [wopool.tile([P, D_], BF, tag="wo", name=f"wo{t}", bufs=NT)
             for t in range(NT)]
    for t in range(NT):
        nc.sync.dma_start(out=wo_sb[t], in_=woT[t * P:(t + 1) * P, :])

    # unnormalized attnT, assembled to full 128-partition tiles via DMA
    au_sb = [aupool.tile([P, SL_], BF, tag="au", name=f"au{t}", bufs=NT)
             for t in range(NT)]
    den_sb = smpool.tile([H_, SL_], BF, tag="den", name="den", bufs=1)

    for t in range(NT):
        # heads 2t (partitions 0-63) and 2t+1 (64-127) interleaved so their
        # K=64 QK matmuls land in disjoint PE row-groups and run concurrently
        ha, hb = HPT * t, HPT * t + 1
        pattn = [pap.tile([65, SL_], F32, tag="pattn", name=f"pa{ha + hf}",
                          bufs=2) for hf in range(HPT)]
        for c in range(KT):
            for n0, n1 in _chunks(SL_, 512):
                w = n1 - n0
                plt = plp.tile([P, 1024], F32, tag="pl", name=f"pl{ha}_{c}_{n0}",
                               bufs=2)
                nc.tensor.matmul(plt[:, 0:w],
                                 lhsT=k_sb[t][0:DEPTH, c * P:(c + 1) * P],
                                 rhs=q_sb[t][0:DEPTH, n0:n1],
                                 start=True, stop=True)
                nc.tensor.matmul(plt[:, w:2 * w],
                                 lhsT=k_sb[t][DEPTH:2 * DEPTH,
                                              c * P:(c + 1) * P],
                                 rhs=q_sb[t][DEPTH:2 * DEPTH, n0:n1],
                                 start=True, stop=True)
                ew = epool.tile([P, 1024], BF, tag="ew", name=f"ew{ha}_{c}_{n0}",
                                bufs=3)
                nc.scalar.activation(ew[:, 0:2 * w], plt[:, 0:2 * w], EXP)
                ew2 = epool.tile([P, 1024], BF, tag="ew2",
                                 name=f"ew2{ha}_{c}_{n0}", bufs=3)
                nc.vector.tensor_mul(ew2[:, 0:w], ew[:, 0:w],
                                     eb_sb[c][:, n0:n1])
                nc.vector.tensor_mul(ew2[:, w:2 * w], ew[:, w:2 * w],
                                     eb_sb[c][:, n0:n1])
                nc.tensor.matmul(pattn[0][:, n0:n1],
                                 lhsT=v_sb[c][:, ha, 0:65],
                                 rhs=ew2[:, 0:w],
                                 start=(c == 0), stop=(c == KT - 1))
                nc.tensor.matmul(pattn[1][:, n0:n1],
                                 lhsT=v_sb[c][:, hb, 0:65],
                                 rhs=ew2[:, w:2 * w],
                                 start=(c == 0), stop=(c == KT - 1))
        for hf in range(HPT):
            h = ha + hf
            r0 = hf * DEPTH
            # denominator row lives at psum partition 64: stage (partition-
            # preserving DVE copy) then SBUF->SBUF DMA into den_sb row h
            stg = stpool.tile([P, SL_], BF, tag="stg", name=f"st{h}", bufs=2)
            nc.vector.tensor_copy(stg[64:65, :], pattn[hf][64:65, :])
            nc.sync.dma_start(out=den_sb[h:h + 1, :], in_=stg[64:65, :])
            # unnormalized attn: DVE to sbuf (base 0) then DMA to row half
            sau = stpool.tile([DEPTH, SL_], BF, tag="sau", name=f"sa{h}",
                              bufs=3)
            nc.vector.tensor_copy(sau, pattn[hf][0:64, :])
            nc.sync.dma_start(out=au_sb[t][r0:r0 + DEPTH, :], in_=sau)

    # ---------------- normalize (in place on au tiles) ----------------
    recipb = smpool.tile([H_, SL_], BF, tag="recip", name="recipb", bufs=1)
    with nc.allow_low_precision(reason="softmax denom reciprocal in bf16"):
        nc.vector.reciprocal(recipb, den_sb)
    # bounce to DRAM: SBUF sources cannot be partition-broadcast, DRAM can
    rscr = dpool.tile([H_, SL_], BF, tag="rscr", name="rscr", bufs=1)
    nc.sync.dma_start(out=rscr, in_=recipb)
    for t in range(NT):
        bc = smpool.tile([P, SL_], BF, tag="bc", name=f"bc{t}", bufs=2)
        for half in range(HPT):
            h = HPT * t + half
            r0 = half * DEPTH
            nc.sync.dma_start(
                out=bc[r0:r0 + DEPTH, :],
                in_=rscr[h:h + 1, :].partition_broadcast(DEPTH))
        nc.vector.tensor_mul(au_sb[t], au_sb[t], bc)

    # ---------------- output projection ----------------
    for m in range(NT):
        osb = opool.tile([P, SL_], F32, tag="osb", name=f"o{m}", bufs=2)
        for n0, n1 in _chunks(SL_, 512):
            ps = plp.tile([P, 1024], F32, tag="pl", name=f"pso{m}_{n0}", bufs=2)
            for t in range(NT):
                nc.tensor.matmul(ps[:, 0:n1 - n0],
                                 lhsT=wo_sb[t][:, m * P:(m + 1) * P],
                                 rhs=au_sb[t][:, n0:n1],
                                 start=(t == 0), stop=(t == NT - 1))
            nc.vector.tensor_copy(osb[:, n0:n1], ps[:, 0:n1 - n0])
        nc.sync.dma_start(out=outT[m * P:(m + 1) * P, :], in_=osb)


def build_nc(S_=S, D_=D, H_=H, SL_=None):
    if SL_ is None:
        SL_ = S_ // 2
    nc = bacc.Bacc("TRN2", target_bir_lowering=False, debug=False)
    io = {
        "xT": nc.dram_tensor("xT", [D_, SL_], BF, kind="ExternalInput").ap(),
        "yT": nc.dram_tensor("yT", [D_, S_], BF, kind="ExternalInput").ap(),
        "ebT": nc.dram_tensor("ebT", [S_, SL_], BF, kind="ExternalInput").ap(),
        "wqT": nc.dram_tensor("wqT", [D_, D_], BF, kind="ExternalInput").ap(),
        "wkT": nc.dram_tensor("wkT", [D_, D_], BF, kind="ExternalInput").ap(),
        "wvT": nc.dram_tensor("wvT", [D_, D_], BF, kind="ExternalInput").ap(),
        "woT": nc.dram_tensor("woT", [D_, D_], BF, kind="ExternalInput").ap(),
        "outT": nc.dram_tensor("outT", [D_, SL_], F32,
                               kind="ExternalOutput").ap(),
    }
    with tile.TileContext(nc) as tc:
        with ExitStack() as ctx:
            _attn_body(ctx, tc, io, S_, D_, H_, SL_)
    nc.compile()
    return nc


_NC_CACHE = None


def kernel(x, y, bias, Wq, Wk, Wv, Wo):
    global _NC_CACHE, last_exec_time_ns, last_results
    x = np.asarray(x, np.float32)
    y = np.asarray(y, np.float32)
    bias = np.asarray(bias, np.float32)
    Wq, Wk, Wv, Wo = (np.asarray(w, np.float32) for w in (Wq, Wk, Wv, Wo))
    SL_ = S // 2
    if _NC_CACHE is None:
        _NC_CACHE = build_nc()
    nc = _NC_CACHE

    bf = ml_dtypes.bfloat16
    scale = DEPTH ** -0.5
    wqT = np.ascontiguousarray(Wq.T * scale).astype(bf)
    wkT = np.ascontiguousarray(Wk.T).astype(bf)
    wvT = np.ascontiguousarray(Wv.T).astype(bf)
    woT = np.ascontiguousarray(Wo.T).astype(bf)
    eb = np.exp(bias[0, 0].astype(np.float32))
    ebT_half = [np.ascontiguousarray(eb[q0:q0 + SL_, :].T).astype(bf)
                for q0 in (0, SL_)]
    yT_all = [np.ascontiguousarray(y[b].T).astype(bf) for b in range(B)]

    in_maps = []
    for core in range(NCORES):
        b, half = divmod(core, 2)
        qs = half * SL_
        in_maps.append({
            "xT": np.ascontiguousarray(x[b, qs:qs + SL_, :].T).astype(bf),
            "yT": yT_all[b],
            "ebT": ebT_half[half],
            "wqT": wqT, "wkT": wkT, "wvT": wvT, "woT": woT,
        })

    res = run_bass_kernel_spmd(nc, in_maps, core_ids=list(range(NCORES)),
                               trace=TRACE)
    last_exec_time_ns = res.exec_time_ns
    last_results = res
    out = np.empty((B, S, D), np.float32)
    for core in range(NCORES):
        b, half = divmod(core, 2)
        qs = half * SL_
        out[b, qs:qs + SL_, :] = res.results[core]["outT"].T
    return out


# revision 11
# speedup vs baseline: 1.4318x; 1.0267x over previous
"""Multi-head attention with bias, distributed over 8 trn2 NeuronCores.

Reference computation (per batch b):
    q = (x @ Wq.T) * depth**-0.5 ; k = y @ Wk.T ; v = y @ Wv.T     (per-head split)
    out = softmax(q @ k.T + bias) @ v @ Wo.T

Sharding: 8 cores = 4 batches x 2 query-row halves.  Core c handles batch
b = c//2 and query rows (c%2)*1024 .. +1024.  k/v projections are computed
redundantly inside each pair (25% extra flops) so there are NO collectives.

Device-side layout (everything "transposed", feature dim on partitions):
    qT/kT = W.T-projected activations [d_out, seq]; v natural [seq, d_out].
    logitsT[kk, i] = kT_h-slice.T @ qT_h-slice  (K=64 contraction)
    expw = exp(logitsT) * exp(bias).T           (exp(bias) precomputed on host)
    attnT_h(+denom row) = [v_h | ones].T @ expw (K=128, denom rides as row 64)
    normalize via batched DVE reciprocal + DMA partition-broadcast from DRAM
    outT = Wo.T-proj of normalized attnT.
Host does: transposes, bf16 casts, exp(bias), scale fold into Wq.
"""

import numpy as np
import ml_dtypes
from contextlib import ExitStack

import concourse.bass as bass
import concourse.mybir as mybir
import concourse.tile as tile
from concourse import bacc
from concourse.bass_utils import run_bass_kernel_spmd

# full-problem dims (hardcoded per spec)
B, S, D, H = 4, 2048, 1024, 16
DEPTH = D // H            # 64
P = 128
NCORES = 8

BF = mybir.dt.bfloat16
F32 = mybir.dt.float32
EXP = mybir.ActivationFunctionType.Exp

TRACE = False
last_exec_time_ns = None
last_results = None


def _chunks(total, step):
    return [(n0, min(n0 + step, total)) for n0 in range(0, total, step)]


def _attn_body(ctx, tc, io, S_, D_, H_, SL_):
    """Emit the per-core kernel.  S_: kv seq len, SL_: q rows on this core."""
    nc = tc.nc
    NT = D_ // P              # d tiles
    KT = S_ // P              # kk tiles
    HPT = P // DEPTH          # heads per d-tile = 2
    xT, yT, ebT, wqT, wkT, wvT, woT, outT = (
        io[k] for k in ("xT", "yT", "ebT", "wqT", "wkT", "wvT", "woT", "outT"))

    # pools that live the whole kernel
    ebpool = ctx.enter_context(tc.tile_pool(name="ebpool", bufs=KT))
    qpool = ctx.enter_context(tc.tile_pool(name="qpool", bufs=2))
    kpool = ctx.enter_context(tc.tile_pool(name="kpool", bufs=2))
    vpool = ctx.enter_context(tc.tile_pool(name="vpool", bufs=KT))
    epool = ctx.enter_context(tc.tile_pool(name="epool", bufs=6))
    aupool = ctx.enter_context(tc.tile_pool(name="aupool", bufs=NT))
    stpool = ctx.enter_context(tc.tile_pool(name="stpool", bufs=3))
    smpool = ctx.enter_context(tc.tile_pool(name="smpool", bufs=4))
    plp = ctx.enter_context(tc.tile_pool(name="plp", bufs=2, space="PSUM"))
    pap = ctx.enter_context(tc.tile_pool(name="pap", bufs=2, space="PSUM"))
    dpool = ctx.enter_context(tc.tile_pool(name="dpool", bufs=1, space="DRAM"))

    v_sb = [vpool.tile([P, H_, 66], BF, tag="v66", name=f"v{c}", bufs=KT)
            for c in range(KT)]
    # unnormalized attnT, assembled to full 128-partition tiles via DMA
    au_sb = [aupool.tile([P, SL_], BF, tag="au", name=f"au{t}", bufs=NT)
             for t in range(NT)]
    den_sb = smpool.tile([H_, SL_], BF, tag="den", name="den", bufs=1)

    # exp(bias) prefetch: consumed from pair 0 on, loads overlap v-proj
    eb_sb = [ebpool.tile([P, SL_], BF, tag="eb", name=f"eb{c}", bufs=KT)
             for c in range(KT)]
    for c in range(KT):
        nc.sync.dma_start(out=eb_sb[c], in_=ebT[c * P:(c + 1) * P, :])

    # ------- v projection up front; q/k projections interleaved with -------
    # ------- attention per head-pair so ACT exp work starts early    -------
    with tc.tile_pool(name="ypool", bufs=NT) as ypool, \
         tc.tile_pool(name="xpool", bufs=NT) as xpool, \
         tc.tile_pool(name="wpool", bufs=2 * NT) as wpool:
        y_sb = [ypool.tile([P, S_], BF, tag="yT", name=f"y{t}", bufs=NT)
                for t in range(NT)]
        for t in range(NT):
            nc.sync.dma_start(out=y_sb[t], in_=yT[t * P:(t + 1) * P, :])
        wv_sb = [wpool.tile([P, D_], BF, tag="w", name=f"wv{t}", bufs=2 * NT)
                 for t in range(NT)]
        for t in range(NT):
            nc.sync.dma_start(out=wv_sb[t], in_=wvT[t * P:(t + 1) * P, :])

        # v in natural layout [kk, head, 66]: cols 0-63 data, 64 ones, 65 pad
        for c in range(KT):
            vt = v_sb[c]
            nc.vector.memset(vt[:, :, 64:65], 1.0)
            nc.vector.memset(vt[:, :, 65:66], 0.0)
            for gi, (n0, n1) in enumerate(_chunks(D_, 512)):
                ps = plp.tile([P, 1024], F32, tag="pl", name=f"psv{c}_{gi}",
                              bufs=2)
                for t in range(NT):
                    nc.tensor.matmul(ps[:, 0:n1 - n0],
                                     lhsT=y_sb[t][:, c * P:(c + 1) * P],
                                     rhs=wv_sb[t][:, n0:n1],
                                     start=(t == 0), stop=(t == NT - 1))
                ng = (n1 - n0) // DEPTH
                src = ps[:, 0:n1 - n0].rearrange("p (g d) -> p g d", d=DEPTH)
                dst = vt[:, gi * ng:(gi + 1) * ng, 0:DEPTH]
                nc.vector.tensor_copy(dst, src)

        x_sb = [xpool.tile([P, SL_], BF, tag="xT", name=f"x{t}", bufs=NT)
                for t in range(NT)]
        for t in range(NT):
            nc.sync.dma_start(out=x_sb[t], in_=xT[t * P:(t + 1) * P, :])
        wq_sb = [wpool.tile([P, D_], BF, tag="w", name=f"wq{t}", bufs=2 * NT)
                 for t in range(NT)]
        for t in range(NT):
            nc.sync.dma_start(out=wq_sb[t], in_=wqT[t * P:(t + 1) * P, :])
        wk_sb = [wpool.tile([P, D_], BF, tag="w", name=f"wk{t}", bufs=2 * NT)
                 for t in range(NT)]
        for t in range(NT):
            nc.sync.dma_start(out=wk_sb[t], in_=wkT[t * P:(t + 1) * P, :])

        for t in range(NT):
            # ---- project q_t / k_t (rotating 2-slot pools) ----
            qt = qpool.tile([P, SL_], BF, tag="qT", name=f"q{t}", bufs=2)
            for n0, n1 in _chunks(SL_, 512):
                ps = plp.tile([P, 1024], F32, tag="pl", name=f"psq{t}_{n0}",
                              bufs=2)
                for u in range(NT):
                    nc.tensor.matmul(ps[:, 0:n1 - n0],
                                     lhsT=wq_sb[u][:, t * P:(t + 1) * P],
                                     rhs=x_sb[u][:, n0:n1],
                                     start=(u == 0), stop=(u == NT - 1))
                nc.vector.tensor_copy(qt[:, n0:n1], ps[:, 0:n1 - n0])
            kt = kpool.tile([P, S_], BF, tag="kT", name=f"k{t}", bufs=2)
            for n0, n1 in _chunks(S_, 512):
                ps = plp.tile([P, 1024], F32, tag="pl", name=f"psk{t}_{n0}",
                              bufs=2)
                for u in range(NT):
                    nc.tensor.matmul(ps[:, 0:n1 - n0],
                                     lhsT=wk_sb[u][:, t * P:(t + 1) * P],
                                     rhs=y_sb[u][:, n0:n1],
                                     start=(u == 0), stop=(u == NT - 1))
                nc.vector.tensor_copy(kt[:, n0:n1], ps[:, 0:n1 - n0])

            # ---- attention for heads 2t (partitions 0-63) and 2t+1 ----
            # (64-127): the two K=64 QK matmuls land in disjoint PE
            # row-groups and run concurrently
            ha, hb = HPT * t, HPT * t + 1
            pattn = [pap.tile([65, SL_], F32, tag="pattn",
                              name=f"pa{ha + hf}", bufs=2)
                     for hf in range(HPT)]
            for c in range(KT):
                for n0, n1 in _chunks(SL_, 512):
                    w = n1 - n0
                    plt = plp.tile([P, 1024], F32, tag="pl",
                                   name=f"pl{ha}_{c}_{n0}", bufs=2)
                    nc.tensor.matmul(plt[:, 0:w],
                                     lhsT=kt[0:DEPTH, c * P:(c + 1) * P],
                                     rhs=qt[0:DEPTH, n0:n1],
                                     start=True, stop=True)
                    nc.tensor.matmul(plt[:, w:2 * w],
                                     lhsT=kt[DEPTH:2 * DEPTH,
                                             c * P:(c + 1) * P],
                                     rhs=qt[DEPTH:2 * DEPTH, n0:n1],
                                     start=True, stop=True)
                    ew = epool.tile([P, 1024], BF, tag="ew",
                                    name=f"ew{ha}_{c}_{n0}", bufs=3)
                    nc.scalar.activation(ew[:, 0:2 * w], plt[:, 0:2 * w], EXP)
                    ew2 = epool.tile([P, 1024], BF, tag="ew2",
                                     name=f"ew2{ha}_{c}_{n0}", bufs=3)
                    nc.vector.tensor_mul(ew2[:, 0:w], ew[:, 0:w],
                                         eb_sb[c][:, n0:n1])
                    nc.vector.tensor_mul(ew2[:, w:2 * w], ew[:, w:2 * w],
                                         eb_sb[c][:, n0:n1])
                    nc.tensor.matmul(pattn[0][:, n0:n1],
                                     lhsT=v_sb[c][:, ha, 0:65],
                                     rhs=ew2[:, 0:w],
                                     start=(c == 0), stop=(c == KT - 1))
                    nc.tensor.matmul(pattn[1][:, n0:n1],
                                     lhsT=v_sb[c][:, hb, 0:65],
                                     rhs=ew2[:, w:2 * w],
                                     start=(c == 0), stop=(c == KT - 1))
            for hf in range(HPT):
                h = ha + hf
                r0 = hf * DEPTH
                # single [65,SL] psum->sbuf copy: rows 0-63 unnormalized
                # attn, row 64 the softmax denominator; then two DMAs
                # scatter it (au row-half + den_sb row h)
                sau = stpool.tile([65, SL_], BF, tag="sau", name=f"sa{h}",
                                  bufs=3)
                nc.vector.tensor_copy(sau, pattn[hf])
                nc.sync.dma_start(out=au_sb[t][r0:r0 + DEPTH, :],
                                  in_=sau[0:64, :])
                nc.sync.dma_start(out=den_sb[h:h + 1, :], in_=sau[64:65, :])

    # ---------------- normalize (in place on au tiles) ----------------
    opool = ctx.enter_context(tc.tile_pool(name="opool", bufs=2))
    wopool = ctx.enter_context(tc.tile_pool(name="wopool", bufs=NT))
    wo_sb = [wopool.tile([P, D_], BF, tag="wo", name=f"wo{t}", bufs=NT)
             for t in range(NT)]
    for t in range(NT):
        nc.sync.dma_start(out=wo_sb[t], in_=woT[t * P:(t + 1) * P, :])

    denf = smpool.tile([H_, SL_], F32, tag="denf", name="denf", bufs=1)
    nc.vector.tensor_copy(denf, den_sb)
    recipf = smpool.tile([H_, SL_], F32, tag="recipf", name="recipf", bufs=1)
    nc.vector.reciprocal_approx_fast(recipf, denf)
    recipb = smpool.tile([H_, SL_], BF, tag="recip", name="recipb", bufs=1)
    nc.vector.tensor_copy(recipb, recipf)
    # bounce to DRAM: SBUF sources cannot be partition-broadcast, DRAM can
    rscr = dpool.tile([H_, SL_], BF, tag="rscr", name="rscr", bufs=1)
    nc.sync.dma_start(out=rscr, in_=recipb)
    for t in range(NT):
        bc = smpool.tile([P, SL_], BF, tag="bc", name=f"bc{t}", bufs=2)
        for half in range(HPT):
            h = HPT * t + half
            r0 = half * DEPTH
            nc.sync.dma_start(
                out=bc[r0:r0 + DEPTH, :],
                in_=rscr[h:h + 1, :].partition_broadcast(DEPTH))
        nc.vector.tensor_mul(au_sb[t], au_sb[t], bc)

    # ---------------- output projection ----------------
    for m in range(NT):
        osb = opool.tile([P, SL_], F32, tag="osb", name=f"o{m}", bufs=2)
        for n0, n1 in _chunks(SL_, 512):
            ps = plp.tile([P, 1024], F32, tag="pl", name=f"pso{m}_{n0}", bufs=2)
            for t in range(NT):
                nc.tensor.matmul(ps[:, 0:n1 - n0],
                                 lhsT=wo_sb[t][:, m * P:(m + 1) * P],
                                 rhs=au_sb[t][:, n0:n1],
                                 start=(t == 0), stop=(t == NT - 1))
            nc.vector.tensor_copy(osb[:, n0:n1], ps[:, 0:n1 - n0])
        nc.sync.dma_start(out=outT[m * P:(m + 1) * P, :], in_=osb)


def build_nc(S_=S, D_=D, H_=H, SL_=None):
    if SL_ is None:
        SL_ = S_ // 2
    nc = bacc.Bacc("TRN2", target_bir_lowering=False, debug=False)
    io = {
        "xT": nc.dram_tensor("xT", [D_, SL_], BF, kind="ExternalInput").ap(),
        "yT": nc.dram_tensor("yT", [D_, S_], BF, kind="ExternalInput").ap(),
        "ebT": nc.dram_tensor("ebT", [S_, SL_], BF, kind="ExternalInput").ap(),
        "wqT": nc.dram_tensor("wqT", [D_, D_], BF, kind="ExternalInput").ap(),
        "wkT": nc.dram_tensor("wkT", [D_, D_], BF, kind="ExternalInput").ap(),
        "wvT": nc.dram_tensor("wvT", [D_, D_], BF, kind="ExternalInput").ap(),
        "woT": nc.dram_tensor("woT", [D_, D_], BF, kind="ExternalInput").ap(),
        "outT": nc.dram_tensor("outT", [D_, SL_], F32,
                               kind="ExternalOutput").ap(),
    }
    with tile.TileContext(nc) as tc:
        with ExitStack() as ctx:
            _attn_body(ctx, tc, io, S_, D_, H_, SL_)
    nc.compile()
    return nc


_NC_CACHE = None


def kernel(x, y, bias, Wq, Wk, Wv, Wo):
    global _NC_CACHE, last_exec_time_ns, last_results
    x = np.asarray(x, np.float32)
    y = np.asarray(y, np.float32)
    bias = np.asarray(bias, np.float32)
    Wq, Wk, Wv, Wo = (np.asarray(w, np.float32) for w in (Wq, Wk, Wv, Wo))
    SL_ = S // 2
    if _NC_CACHE is None:
        _NC_CACHE = build_nc()
    nc = _NC_CACHE

    bf = ml_dtypes.bfloat16
    scale = DEPTH ** -0.5
    wqT = np.ascontiguousarray(Wq.T * scale).astype(bf)
    wkT = np.ascontiguousarray(Wk.T).astype(bf)
    wvT = np.ascontiguousarray(Wv.T).astype(bf)
    woT = np.ascontiguousarray(Wo.T).astype(bf)
    eb = np.exp(bias[0, 0].astype(np.float32))
    ebT_half = [np.ascontiguousarray(eb[q0:q0 + SL_, :].T).astype(bf)
                for q0 in (0, SL_)]
    yT_all = [np.ascontiguousarray(y[b].T).astype(bf) for b in range(B)]

    in_maps = []
    for core in range(NCORES):
        b, half = divmod(core, 2)
        qs = half * SL_
        in_maps.append({
            "xT": np.ascontiguousarray(x[b, qs:qs + SL_, :].T).astype(bf),
            "yT": yT_all[b],
            "ebT": ebT_half[half],
            "wqT": wqT, "wkT": wkT, "wvT": wvT, "woT": woT,
        })

    res = run_bass_kernel_spmd(nc, in_maps, core_ids=list(range(NCORES)),
                               trace=TRACE)
    last_exec_time_ns = res.exec_time_ns
    last_results = res
    out = np.empty((B, S, D), np.float32)
    for core in range(NCORES):
        b, half = divmod(core, 2)
        qs = half * SL_
        out[b, qs:qs + SL_, :] = res.results[core]["outT"].T
    return out
